# revision 27
# baseline (speedup 1.0000x reference)
"""Trainium2 Bass kernel for nn_CrossModalAttention (B=4, LQ=1024, LKV=2048,
QDIM=1024, KDIM=VDIM=768, ODIM=1024, H=16, HD=64) on 8 NeuronCores.

Sharding: core c -> batch b = c//2, head-group g = c%2 (8 heads = 512 odim cols
of Wq/Wk/Wv, 512 rows of Wo). Each core computes a PARTIAL output projection
(its 8 heads' contribution to all 1024 output cols, bf16); the host sums the
two partials per batch and adds bo. No on-device collectives.

KV mask compression: the boolean mask is a host-visible input, so the host
gathers only the unmasked key/value rows (exactly equivalent: masked positions
contribute exp(-inf)=0 to numerator and denominator), pads to a multiple of
128, and passes a per-position bias (0 valid / -1e5 pad). This halves K/V
projections, scores, exp and AV for ~50% masks. The NEFF is compiled per
padded-chunk-count and cached.

Compute dtype: bf16 matmuls with fp32 PSUM accumulation. Host-side sharding
casts to bf16 and pre-transposes activations to contraction-dim-major.

Dataflow per core (activations kept transposed):
  qT[512,1024]  = Wq_g^T chunks @ queryT
  kT[512,KVC]   = Wk_g^T chunks @ keyT
  v[KVC,520]    = valueT chunks @ Wv_g        (+ ones column per head)
  S^T[128,1024] = kT_h chunk (stationary, K=64) @ qT_h
  P^T           = exp(S^T/8 + pad_bias)       (ACT)
  A'^T[65,1024] = [v_h | 1] (stationary) @ P^T  -> row 64 = softmax denominator
  A^T           = A'^T[0:64] * (1/denom)
  part[1024,1024] = sum_hp A^T[hp] chunks (stationary) @ Wo[g-block rows]

The PE issues matmuls serially at ~0.42ns/col regardless of K/M, so the
schedule's only job is keeping the tensor queue fed: K-proj starts as soon as
wk + the first keyT piece land, and q/o-proj matmuls are injected into the
scalar(exp)-bound attention chunk loops.
"""

import os
import numpy as np

import concourse.bass as bass
import concourse.mybir as mybir
import concourse.tile as tile
from concourse import bacc
from concourse import bass_utils

F32 = mybir.dt.float32
BF16 = mybir.dt.bfloat16

B, LQ, LKV = 4, 1024, 2048
QDIM, KDIM, ODIM, H, HD = 1024, 768, 1024, 16, 64
OD_L = 512            # odim per core (8 heads)
QK = QDIM // 128      # 8  qdim chunks
KK = KDIM // 128      # 6  kdim chunks
MT = OD_L // 128      # 4  local odim tiles (= head pairs)
N_CORES = 8
NEG_BIG = -100000.0


def _col_pieces(total, piece=512):
    out = []
    c = 0
    while c < total:
        w = min(piece, total - c)
        out.append((c, w))
        c += w
    return out


def _emit(nc, tc, ltc):
    AF = mybir.ActivationFunctionType
    KVC = ltc * 128

    qt_d = nc.dram_tensor("qt", [QDIM, LQ], BF16, kind="ExternalInput")
    kt_d = nc.dram_tensor("kt", [KDIM, KVC], BF16, kind="ExternalInput")
    vt_d = nc.dram_tensor("vt", [KDIM, KVC], BF16, kind="ExternalInput")
    mb_d = nc.dram_tensor("maskb", [128, ltc], F32, kind="ExternalInput")
    wq_d = nc.dram_tensor("wq", [QDIM, OD_L], BF16, kind="ExternalInput")
    wk_d = nc.dram_tensor("wk", [KDIM, OD_L], BF16, kind="ExternalInput")
    wv_d = nc.dram_tensor("wv", [KDIM, OD_L], BF16, kind="ExternalInput")
    wo_d = nc.dram_tensor("wo", [OD_L, ODIM], BF16, kind="ExternalInput")
    bq_d = nc.dram_tensor("bq", [OD_L], F32, kind="ExternalInput")
    bk_d = nc.dram_tensor("bk", [OD_L], F32, kind="ExternalInput")
    bv_d = nc.dram_tensor("bv", [OD_L], F32, kind="ExternalInput")
    out_d = nc.dram_tensor("out", [LQ, ODIM], BF16, kind="ExternalOutput")

    with (
        tc.tile_pool(name="const", bufs=1) as cp,
        tc.tile_pool(name="act", bufs=1) as ap_,
        tc.tile_pool(name="pt", bufs=4) as ptp,
        tc.tile_pool(name="small", bufs=1) as smp,
        tc.tile_pool(name="psum", bufs=2, space="PSUM") as pp,
    ):
        # ---- loads, ordered by first need, spread over 4 queues -----------
        # sync q: keyT pieces then queryT halves (gates k_proj / q_proj)
        # scalar q: wk first (gates k_proj), small consts, wq, wv
        # gpsimd q: valueT pieces then wo
        # vector q: second queryT half (parallel with sync)
        wk_sb = cp.tile([128, KK, OD_L], BF16, name="wk_sb")
        wk3 = wk_d.ap().rearrange("(k p) c -> p k c", p=128)
        for k0 in range(0, KK, 2):
            nc.scalar.dma_start(out=wk_sb[:, k0:k0 + 2, :], in_=wk3[:, k0:k0 + 2, :])
        keyT = cp.tile([128, KK, KVC], BF16, name="keyT")
        kt3 = kt_d.ap().rearrange("(k p) l -> p k l", p=128)
        p0w = min(256, KVC)
        kpieces = [(0, p0w)] + [(p0w + c0, w) for c0, w in _col_pieces(KVC - p0w)]
        queryT = cp.tile([128, QK, LQ], BF16, name="queryT")
        qt3 = qt_d.ap().rearrange("(k p) l -> p k l", p=128)
        # sync queue: qt half 0 first (the hp0 loop gates on the full qT
        # projection), then kt pieces in scores-consumption order
        nc.sync.dma_start(out=queryT[:, :, 0:512], in_=qt3[:, :, 0:512])
        nc.sync.dma_start(out=keyT[:, :, 0:p0w], in_=kt3[:, :, 0:p0w])
        for c0, w in kpieces[1:]:
            nc.sync.dma_start(out=keyT[:, :, c0:c0 + w], in_=kt3[:, :, c0:c0 + w])

        wq_sb = cp.tile([128, QK, OD_L], BF16, name="wq_sb")
        wq3 = wq_d.ap().rearrange("(k p) c -> p k c", p=128)
        nc.scalar.dma_start(out=wq_sb[:, :, 0:128], in_=wq3[:, :, 0:128])
        bqc = cp.tile([128, MT], F32, name="bqc")
        bkc = cp.tile([128, MT], F32, name="bkc")
        nc.scalar.dma_start(out=bqc[:], in_=bq_d.ap().rearrange("(m p) -> p m", p=128))
        nc.scalar.dma_start(out=bkc[:], in_=bk_d.ap().rearrange("(m p) -> p m", p=128))
        maskb = cp.tile([128, ltc], F32, name="maskb")
        nc.scalar.dma_start(out=maskb[:], in_=mb_d.ap())
        bv_row = smp.tile([1, OD_L], F32, name="bv_row", tag="bv_row")
        nc.scalar.dma_start(out=bv_row[:], in_=bv_d.ap())
        wv_sb = cp.tile([128, KK, OD_L], BF16, name="wv_sb")
        nc.scalar.dma_start(out=wv_sb[:], in_=wv_d.ap().rearrange("(k p) c -> p k c", p=128))
        nc.scalar.dma_start(out=wq_sb[:, :, 128:OD_L], in_=wq3[:, :, 128:OD_L])

        # gpsimd queue: second query half first (gates the hp0 loop), then
        # value pieces in v_proj order, then wo (first needed mid-loop-1)
        valueT = cp.tile([128, KK, KVC], BF16, name="valueT")
        vt3 = vt_d.ap().rearrange("(k p) l -> p k l", p=128)
        nc.gpsimd.dma_start(out=queryT[:, :, 512:1024], in_=qt3[:, :, 512:1024])
        for c0, w in kpieces:
            nc.gpsimd.dma_start(out=valueT[:, :, c0:c0 + w], in_=vt3[:, :, c0:c0 + w])
        wo_sb = cp.tile([128, MT, ODIM], BF16, name="wo_sb")
        nc.gpsimd.dma_start(out=wo_sb[:], in_=wo_d.ap().rearrange("(m p) c -> p m c", p=128))

        bv_b = cp.tile([128, OD_L], F32, name="bv_b")
        nc.gpsimd.partition_broadcast(bv_b[:], bv_row[:])

        # ---- persistent activation tensors -------------------------------
        qT_sb = ap_.tile([128, MT, LQ], BF16, name="qT_sb")
        kT_sb = ap_.tile([128, MT, KVC], BF16, name="kT_sb")
        v_sb = ap_.tile([128, ltc, 8, HD + 1], BF16, name="v_sb")
        atT_sb = ap_.tile([128, MT, LQ], BF16, name="atT_sb")
        out_acc = ap_.tile([128, 8, ODIM], F32, name="out_acc")
        out_bf = ap_.tile([128, 8, ODIM], BF16, name="out_bf")

        # ones column for the softmax denominator (pads neutralized by bias)
        nc.gpsimd.memset(v_sb[:, :, :, HD:HD + 1], 1.0)

        def q_proj_nt(mt, nt):
            ps = pp.tile([128, 512], F32, name="ps_proj", tag="s")
            for k in range(QK):
                nc.tensor.matmul(
                    ps[:],
                    lhsT=wq_sb[:, k, mt * 128:(mt + 1) * 128],
                    rhs=queryT[:, k, nt * 512:(nt + 1) * 512],
                    start=(k == 0), stop=(k == QK - 1),
                )
            nc.vector.tensor_scalar_add(
                qT_sb[:, mt, nt * 512:(nt + 1) * 512], ps[:], bqc[:, mt:mt + 1])

        def k_proj_piece(mt, c0, w):
            ps = pp.tile([128, 512], F32, name="ps_proj", tag="s")
            for k in range(KK):
                nc.tensor.matmul(
                    ps[:, 0:w],
                    lhsT=wk_sb[:, k, mt * 128:(mt + 1) * 128],
                    rhs=keyT[:, k, c0:c0 + w],
                    start=(k == 0), stop=(k == KK - 1),
                )
            nc.vector.tensor_scalar_add(
                kT_sb[:, mt, c0:c0 + w], ps[:, 0:w], bkc[:, mt:mt + 1])

        def v_proj(lt):
            ps = pp.tile([128, 512], F32, name="ps_proj", tag="s")
            for k in range(KK):
                nc.tensor.matmul(
                    ps[:],
                    lhsT=valueT[:, k, lt * 128:(lt + 1) * 128],
                    rhs=wv_sb[:, k, :],
                    start=(k == 0), stop=(k == KK - 1),
                )
            nc.vector.tensor_add(
                v_sb[:, lt, :, 0:HD],
                ps[:].rearrange("p (a d) -> p a d", a=8),
                bv_b[:].rearrange("p (a d) -> p a d", a=8),
            )

        out_q = [nc.sync, nc.scalar, nc.gpsimd]

        def o_proj_lqm(hp, lqm):
            # head-pair hp's contribution to the partial output projection
            for nt in range(2):
                po = pp.tile([128, 512], F32, name="po", tag="s")
                nc.tensor.matmul(
                    po[:],
                    lhsT=atT_sb[:, hp, lqm * 128:(lqm + 1) * 128],
                    rhs=wo_sb[:, hp, nt * 512:(nt + 1) * 512],
                    start=True, stop=True,
                )
                sl = slice(nt * 512, (nt + 1) * 512)
                if hp == 0:
                    nc.vector.tensor_copy(out_acc[:, lqm, sl], po[:])
                elif hp < MT - 1:
                    nc.vector.tensor_add(out_acc[:, lqm, sl], po[:], out_acc[:, lqm, sl])
                else:
                    nc.vector.tensor_add(out_bf[:, lqm, sl], po[:], out_acc[:, lqm, sl])
            if hp == MT - 1:
                out_q[lqm % 3].dma_start(
                    out=out_d[lqm * 128:(lqm + 1) * 128, :], in_=out_bf[:, lqm, :])

        # ---- PE warmup: dummy matmuls during the head DMA window so the
        # HAM clock-gate releases (1.2 -> 2.4 GHz) before real work lands.
        # memset on the (empty) vector queue so nothing delays them.
        dmy = cp.tile([128, 512], BF16, name="dmy")
        nc.vector.memset(dmy[:], 0.0)
        for _ in range(12):
            ps = pp.tile([128, 512], F32, name="ps_proj", tag="s")
            nc.tensor.matmul(ps[:], lhsT=dmy[:, 0:128], rhs=dmy[:], start=True, stop=True)

        # ---- head phase: only what gates the hp0 chunk loop's first two
        # chunks (kt piece0 covers chunks 0-1; later pieces are injected)
        k_proj_piece(0, *kpieces[0])
        q_proj_nt(0, 0)
        q_proj_nt(0, 1)

        # Deferred tensor work. slots[hp][c] = thunks emitted after chunk
        # c's exp in loop hp (so they never delay the scores->exp critical
        # path); bdry[hp] = small burst right after loop hp. Placement
        # rules: v_proj(c) by chunk c (consumer attn_v(c) is one chunk
        # later); k_proj(m)/q_proj(m) complete before loop m starts;
        # o_proj(hp) goes into loop hp+1 once normalize(hp) has drained
        # (chunk 2+), spreading its vector psum-drains across the loop.
        slots = [[[] for _ in range(ltc)] for _ in range(MT)]
        bdry = [[] for _ in range(MT)]

        def put(hp, c, thunk):
            slots[hp][max(0, min(c, ltc - 1))].append(thunk)

        for lt in range(ltc):
            put(0, lt, lambda l=lt: v_proj(l))
        for c0, w in kpieces[1:]:
            put(0, c0 // 128 - 1, lambda c0=c0, w=w: k_proj_piece(0, c0, w))
        for j, (c0, w) in enumerate(kpieces):
            put(0, ltc - 3 + j, lambda c0=c0, w=w: k_proj_piece(1, c0, w))
        bdry[0] = [lambda n=n: q_proj_nt(1, n) for n in (0, 1)]
        for hp in range(1, MT):
            if hp + 1 < MT:
                for j, (c0, w) in enumerate(kpieces):
                    put(hp, j, lambda m=hp + 1, c0=c0, w=w: k_proj_piece(m, c0, w))
                bdry[hp] = [lambda m=hp + 1, n=n: q_proj_nt(m, n) for n in (0, 1)]
            for m in range(8):
                put(hp, 2 + m * (ltc - 2) // 8, lambda h=hp - 1, m=m: o_proj_lqm(h, m))

        # ---- attention ----------------------------------------------------
        for hp in range(MT):
            av_a = pp.tile([HD + 1, LQ], F32, name="av_a", tag="av")
            av_b = pp.tile([HD + 1, LQ], F32, name="av_b", tag="av")

            def attn_v(c, pt_a, pt_b):
                for nt in range(2):
                    nc.tensor.matmul(
                        av_a[:, nt * 512:(nt + 1) * 512],
                        lhsT=v_sb[:, c, 2 * hp, :],
                        rhs=pt_a[:, nt * 512:(nt + 1) * 512],
                        start=(c == 0), stop=(c == ltc - 1),
                    )
                    nc.tensor.matmul(
                        av_b[:, nt * 512:(nt + 1) * 512],
                        lhsT=v_sb[:, c, 2 * hp + 1, :],
                        rhs=pt_b[:, nt * 512:(nt + 1) * 512],
                        start=(c == 0), stop=(c == ltc - 1),
                    )

            pt_prev = None
            for c in range(ltc):
                s_a = pp.tile([128, LQ], F32, name="s_a", tag="s")
                s_b = pp.tile([128, LQ], F32, name="s_b", tag="s")
                for nt in range(2):
                    nc.tensor.matmul(
                        s_a[:, nt * 512:(nt + 1) * 512],
                        lhsT=kT_sb[0:64, hp, c * 128:(c + 1) * 128],
                        rhs=qT_sb[0:64, hp, nt * 512:(nt + 1) * 512],
                        tile_position=(0, 0),
                    )
                    nc.tensor.matmul(
                        s_b[:, nt * 512:(nt + 1) * 512],
                        lhsT=kT_sb[64:128, hp, c * 128:(c + 1) * 128],
                        rhs=qT_sb[64:128, hp, nt * 512:(nt + 1) * 512],
                        tile_position=(64, 0),
                    )
                pt_a = ptp.tile([128, LQ], BF16, name="pt_a", tag="pt")
                pt_b = ptp.tile([128, LQ], BF16, name="pt_b", tag="pt")
                nc.scalar.activation(pt_a[:], s_a[:], AF.Exp,
                                     bias=maskb[:, c:c + 1], scale=0.125)
                nc.scalar.activation(pt_b[:], s_b[:], AF.Exp,
                                     bias=maskb[:, c:c + 1], scale=0.125)
                # deferred proj work AFTER the exp emission so it never
                # delays the scores->exp critical path in the tensor queue
                for thunk in slots[hp][c]:
                    thunk()
                if pt_prev is not None:
                    attn_v(c - 1, *pt_prev)
                pt_prev = (pt_a, pt_b)
            attn_v(ltc - 1, *pt_prev)

            # boundary burst first: independent tensor work (and its vector
            # psum-drains) must be queued AHEAD of the normalize chain so
            # the chain's serialization doesn't idle the PE
            for thunk in bdry[hp]:
                thunk()

            # softmax normalize: 1/denominator (row HD of av) broadcast to
            # the 64 head rows, applied to av. The custom-DVE reciprocal
            # reads garbage from PSUM on real HW (sim divergence) — stage
            # the denominator rows through SBUF, on the idle scalar engine.
            dsb_a = smp.tile([1, LQ], F32, name="dsb_a", tag="dsb_a")
            dsb_b = smp.tile([1, LQ], F32, name="dsb_b", tag="dsb_b")
            nc.scalar.copy(dsb_a[:], av_a[HD:HD + 1, :])
            nc.scalar.copy(dsb_b[:], av_b[HD:HD + 1, :])
            rec_a = smp.tile([1, LQ], F32, name="rec_a", tag="rec_a")
            rec_b = smp.tile([1, LQ], F32, name="rec_b", tag="rec_b")
            nc.vector.reciprocal_approx_fast(rec_a[:], dsb_a[:])
            nc.vector.reciprocal_approx_fast(rec_b[:], dsb_b[:])
            rb_a = smp.tile([64, LQ], F32, name="rb_a", tag="rb_a")
            rb_b = smp.tile([64, LQ], F32, name="rb_b", tag="rb_b")
            nc.gpsimd.partition_broadcast(rb_a[:], rec_a[:])
            nc.gpsimd.partition_broadcast(rb_b[:], rec_b[:])
            nc.vector.tensor_mul(atT_sb[0:64, hp, :], av_a[0:HD, :], rb_a[:])
            nc.vector.tensor_mul(atT_sb[64:128, hp, :], av_b[0:HD, :], rb_b[:])

        # ---- tail: last head-pair's output contribution ------------------
        for lqm in range(8):
            o_proj_lqm(MT - 1, lqm)


_NC_CACHE = {}


def _build(ltc):
    if ltc in _NC_CACHE:
        return _NC_CACHE[ltc]
    nc = bacc.Bacc("TRN2", target_bir_lowering=False, debug=False,
                   num_devices=N_CORES)
    with tile.TileContext(nc) as tc:
        _emit(nc, tc, ltc)
    nc.compile()
    _NC_CACHE[ltc] = nc
    return nc


def _shard_inputs(inputs):
    import ml_dtypes
    BF = ml_dtypes.bfloat16

    def bf(x):
        return np.ascontiguousarray(np.asarray(x, dtype=np.float32).astype(BF))

    mask = np.asarray(inputs["mask"])  # [B, LKV], True = masked out
    keep = [np.nonzero(~mask[b])[0] for b in range(B)]
    ltc = max(1, max((len(k) + 127) // 128 for k in keep))
    kvc = ltc * 128

    qT, kT, vT, mbs = [], [], [], []
    key_f = np.asarray(inputs["key"], dtype=np.float32)
    val_f = np.asarray(inputs["value"], dtype=np.float32)
    for b in range(B):
        idx = keep[b]
        nk = len(idx)
        kc = np.zeros((kvc, KDIM), dtype=np.float32)
        vc = np.zeros((kvc, KDIM), dtype=np.float32)
        kc[:nk] = key_f[b][idx]
        vc[:nk] = val_f[b][idx]
        qT.append(bf(np.asarray(inputs["query"][b], dtype=np.float32).T))
        kT.append(bf(kc.T))
        vT.append(bf(vc.T))
        mb = np.zeros((kvc,), dtype=np.float32)
        mb[nk:] = NEG_BIG
        mbs.append(np.ascontiguousarray(mb.reshape(ltc, 128).T))

    Wq, Wk = bf(inputs["Wq"]), bf(inputs["Wk"])
    Wv, Wo = bf(inputs["Wv"]), bf(inputs["Wo"])
    bq = np.asarray(inputs["bq"], dtype=np.float32)
    bk = np.asarray(inputs["bk"], dtype=np.float32)
    bv = np.asarray(inputs["bv"], dtype=np.float32)
    in_maps = []
    for c in range(N_CORES):
        b, g = c // 2, c % 2
        sl = slice(g * OD_L, (g + 1) * OD_L)
        in_maps.append({
            "qt": qT[b], "kt": kT[b], "vt": vT[b], "maskb": mbs[b],
            "wq": np.ascontiguousarray(Wq[:, sl]),
            "wk": np.ascontiguousarray(Wk[:, sl]),
            "wv": np.ascontiguousarray(Wv[:, sl]),
            "wo": np.ascontiguousarray(Wo[sl, :]),
            "bq": np.ascontiguousarray(bq[sl]),
            "bk": np.ascontiguousarray(bk[sl]),
            "bv": np.ascontiguousarray(bv[sl]),
        })
    return in_maps, ltc


def _install_trace_hooks():
    """Best-effort NTFF profiling hooks for axon (used only when tracing)."""
    import sys, types
    try:
        from antenv.axon_hooks import get_axon_ntff_profile_hook  # noqa: F401
        return
    except Exception:
        pass
    try:
        from trn_agent_boot.trn_boot import _ntff_profile_via_ctypes
        hook = _ntff_profile_via_ctypes("/opt/axon/libaxon_pjrt.so")
        mod = types.ModuleType("antenv.axon_hooks")
        mod.get_axon_ntff_profile_hook = lambda: hook
        mod.set_axon_ntff_profile_hook = lambda h: None
        sys.modules["antenv.axon_hooks"] = mod
        import antenv
        antenv.axon_hooks = mod
    except Exception as e:  # pragma: no cover
        print(f"trace hook install failed: {e}")
    # avoid S3 uploads from the profile path
    bass_utils.upload_artifacts = lambda tmpdir: tmpdir


last_exec_time_ns = None
last_trace_dir = None


def kernel(**inputs) -> np.ndarray:
    global last_exec_time_ns, last_trace_dir
    trace = os.environ.get("KERNEL_TRACE", "0") == "1"
    in_maps, ltc = _shard_inputs(inputs)
    nc = _build(ltc)
    kwargs = {}
    if trace:
        _install_trace_hooks()
        import tempfile
        tmpdir = tempfile.mkdtemp(prefix="xmattn_trace_")
        kwargs = dict(trace=True, tmpdir=tmpdir, trace_cores=[0])
        last_trace_dir = tmpdir
    res = bass_utils.run_bass_kernel_spmd(
        nc, in_maps, core_ids=list(range(N_CORES)), **kwargs)
    last_exec_time_ns = res.exec_time_ns
    bo = np.asarray(inputs["bo"], dtype=np.float32)
    out = np.empty((B, LQ, ODIM), dtype=np.float32)
    for b in range(B):
        p0 = np.asarray(res.results[2 * b]["out"], dtype=np.float32)
        p1 = np.asarray(res.results[2 * b + 1]["out"], dtype=np.float32)
        out[b] = p0 + p1 + bo
    return out


if __name__ == "__main__":
    d = np.load(os.path.join(os.path.dirname(__file__), "ref_data.npz"))
    inputs = {k: d[k] for k in d.files if k != "expected"}
    got = kernel(**inputs)
    exp = d["expected"]
    rel = np.linalg.norm(got - exp) / np.linalg.norm(exp)
    print("Relative error:", rel)
    print("HW exec time:", last_exec_time_ns, "ns")
